# revision 1
# baseline (speedup 1.0000x reference)
"""VQ codebook soft-assignment encoding kernel for 8 trn2 NeuronCores.

Math (per batch b):
  Xf = X[b].reshape(D, N).T                        # [N, D], N = H*W
  logit[n,k] = scale[k] * (||x_n||^2 - 2<x_n,c_k> + ||c_k||^2)
  A = softmax(logit, axis=k)
  E[b,k,:] = sum_n A[n,k] * (x_n - c_k)            # [K, D]

Sharding: data-parallel over B (4 batches per core), codewords/scale replicated.

Device plan per core (all heavy compute in fp16 with fp32 PSUM accumulation):
  - SWDGE cast-load X[b] fp32 HBM -> fp16 SBUF, native [d, n] layout
  - logits in [k, n] layout:  PSUM = G^T X + S^T X^2  where G[d,k] = -2 s_k c[k,d],
    S[d,k] = s_k  (so S^T X^2 contributes s_k*||x_n||^2);  then
    U = exp(PSUM + bias_k),  bias_k = s_k ||c_k||^2  (ACT, per-partition bias)
  - DMA-xbar transposes (fp16): X -> XT [128, 72, 128] tiles ([n-part, d])
  - U -> UT via DVE stream-transpose (4x 32-partition instructions), keeping
    the scheduler's global DMA fence chain free of the U transpose
  - normalize on DVE: den = reduce_k UT, A = UT * (1/den)
  - E-matmul on PE: psE[32, 257] += A_t^T @ XT_t over 72 n-chunks
    -> cols 0:256 = sum_n A[n,k] x[n,d], col 256 = S_k = sum_n A[n,k]
  - E = psE[:, :256] - S_k * c  (DVE), DMA out fp32
"""
import numpy as np
from contextlib import ExitStack

import concourse.bass as bass
import concourse.mybir as mybir
import concourse.tile as tile
from concourse.tile import ScopedClock
from concourse.bass_utils import run_bass_kernel_spmd

dt = mybir.dt

B, D, K, H, W = 32, 256, 32, 96, 96
N = H * W                 # 9216
NCORES = 8
BPC = B // NCORES         # 4 batches per core
TN = 512                  # n-tile for logits pass
NT = N // TN              # 18
NCHUNK = N // 128         # 72 e-matmul chunks
SQG = 3                   # n-tiles per square group

USE_DVE_UT = True         # U transpose on DVE instead of DMA xbar


def _patch_tile_drain():
    """This toolchain's walrus allows only one sync-wait per instruction.
    Split the tail drain's waits across chained drains."""
    if getattr(tile.TileContext, "_drain_patched", False):
        return

    def _drain_and_barrier_split(self, tick_clock, wait_clock):
        nc = self.nc
        drain_inst = nc.sync.drain()
        wait_clock.add_sem_waits(
            drain_inst.ins, ScopedClock({None: tick_clock.global_clock})
        )
        si = drain_inst.ins.sync_info
        if si is not None and si.on_wait and len(si.on_wait) > 1:
            extra = list(si.on_wait[1:])
            del si.on_wait[1:]
            for w in extra:
                d = nc.sync.drain()
                dsi = d.ins.sync_info
                if dsi is None:
                    d.ins.sync_info = mybir.SyncInfo(on_wait=[w], on_update=[])
                else:
                    dsi.on_wait.append(w)
        nc.all_engine_barrier()
        assert self.sems is not None
        popped = nc._tile_sem_poison_stack.pop()
        assert popped is self._sem_poison
        nc.clear_and_free_semaphores(list(self.sems.allocated().values()))
        nc.all_engine_barrier()

    tile.TileContext._drain_and_barrier = _drain_and_barrier_split
    tile.TileContext._drain_patched = True


def _split_multi_waits(nc):
    """Hoist extra sem-waits onto standalone event-sem instructions."""
    n_split = 0
    for f in nc.m.functions:
        for bb in f.blocks:
            new_list = []
            for inst in bb.instructions:
                si = inst.sync_info
                if si is not None and si.on_wait is not None and len(si.on_wait) > 1:
                    extra = list(si.on_wait[:-1])
                    keep = [si.on_wait[-1]]
                    for w in extra:
                        ev = mybir.InstEventSemaphore(
                            name=f"{inst.name}-wsplit{n_split}",
                            ins=[], outs=[],
                            sync_info=mybir.SyncInfo(on_wait=[w], on_update=[]),
                        )
                        ev.engine = inst.engine
                        nc.register_instruction(ev)
                        new_list.append(ev)
                        n_split += 1
                    del si.on_wait[:]
                    si.on_wait.extend(keep)
                new_list.append(inst)
            bb.instructions[:] = new_list
    return n_split


def _build_module():
    _patch_tile_drain()
    nc = bass.Bass()
    xin = nc.declare_dram_parameter("xin", [BPC, D, N], dt.float32, isOutput=False)
    cw = nc.declare_dram_parameter("cw", [K, D], dt.float32, isOutput=False)
    s_col = nc.declare_dram_parameter("s_col", [K, 1], dt.float32, isOutput=False)
    s_row = nc.declare_dram_parameter("s_row", [1, K], dt.float32, isOutput=False)
    eout = nc.declare_dram_parameter("eout", [BPC, K, D], dt.float32, isOutput=True)

    f16, f32 = dt.float16, dt.float32
    AX = mybir.AxisListType.X
    EXP = mybir.ActivationFunctionType.Exp

    with tile.TileContext(nc) as tc:
        with ExitStack() as ctx:
            singles = ctx.enter_context(tc.tile_pool(name="singles", bufs=1))
            psprep = ctx.enter_context(tc.tile_pool(name="psprep", bufs=1, space="PSUM"))

            # ---- one-time prep from codewords/scale ----
            cw_sb = singles.tile([K, D], f32)
            nc.sync.dma_start(cw_sb[:], cw[:])
            scol_sb = singles.tile([K, 1], f32)
            nc.sync.dma_start(scol_sb[:], s_col[:])
            srow_sb = singles.tile([1, K], f32)
            nc.sync.dma_start(srow_sb[:], s_row[:])

            # G16 [128, 2, K]: G[p, c, k] = -2 s_k c[k, c*128+p]
            w1 = singles.tile([K, D], f32)
            nc.vector.tensor_scalar_mul(w1[:], cw_sb[:], scol_sb[:])
            w2 = singles.tile([K, D], f32)
            nc.vector.tensor_scalar_mul(w2[:], w1[:], -2.0)
            w16 = singles.tile([K, D], f16)
            nc.vector.tensor_copy(w16[:], w2[:])
            g16 = singles.tile([128, 2 * K], f16)
            # g16[32j+w, c, k] = w16[k, c*128+32j+w] via DVE 32x32-block
            # transposes (keeps the prep transpose off the DMA fence chain)
            g3t = g16[:].rearrange("p (c k) -> p c k", k=K)
            wv = w16[:].rearrange("k (c j w) -> k c j w", j=4, w=32)
            for j in range(4):
                nc.vector.transpose(g3t[32 * j:32 * (j + 1), :, :], wv[:, :, j, :])

            # S16 [128, K]: every row = s_k (fp16)
            ones_row16 = singles.tile([1, 128], f16)
            nc.vector.memset(ones_row16[:], 1.0)
            srow16 = singles.tile([1, K], f16)
            nc.vector.tensor_copy(srow16[:], srow_sb[:])
            ps_s = psprep.tile([128, K], f32)
            nc.tensor.matmul(ps_s[:], ones_row16[:], srow16[:], start=True, stop=True,
                             skip_group_check=True)
            s16 = singles.tile([128, K], f16)
            nc.vector.tensor_copy(s16[:], ps_s[:])

            # bias [K, 1] = s_k * ||c_k||^2
            csq = singles.tile([K, D], f32)
            nc.vector.tensor_mul(csq[:], cw_sb[:], cw_sb[:])
            sqc = singles.tile([K, 1], f32)
            nc.vector.reduce_sum(
                sqc[:].rearrange("k (o p) -> k o p", o=1),
                csq[:].rearrange("k (o d) -> k o d", o=1), axis=AX)
            bias = singles.tile([K, 1], f32)
            nc.vector.tensor_mul(bias[:], sqc[:], scol_sb[:])

            ones_col16 = singles.tile([128, 1], f16)
            nc.vector.memset(ones_col16[:], 1.0)

            # ---- per-batch pools ----
            xpool = ctx.enter_context(tc.tile_pool(name="x16", bufs=2))
            xtpool = ctx.enter_context(tc.tile_pool(name="xt", bufs=2))
            sqpool = ctx.enter_context(tc.tile_pool(name="xsq", bufs=2))
            upool = ctx.enter_context(tc.tile_pool(name="u16", bufs=1))
            utpool = ctx.enter_context(tc.tile_pool(name="ut", bufs=2))
            apool = ctx.enter_context(tc.tile_pool(name="a16", bufs=2))
            npool = ctx.enter_context(tc.tile_pool(name="nrm", bufs=3))
            opool = ctx.enter_context(tc.tile_pool(name="out", bufs=1))
            psl = ctx.enter_context(tc.tile_pool(name="psl", bufs=3, space="PSUM"))
            pse = ctx.enter_context(tc.tile_pool(name="pse", bufs=2, space="PSUM"))

            efs = []
            for b in range(BPC):
                x0 = xpool.tile([128, N], f16, tag="x0")
                nc.gpsimd.dma_start(x0[:], xin[b, 0:128, :])
                x1 = xpool.tile([128, N], f16, tag="x1")
                nc.gpsimd.dma_start(x1[:], xin[b, 128:256, :])

                xt0 = xtpool.tile([128, NCHUNK * 128], f16, tag="xt0")
                xt0v = xt0[:].rearrange("p (t w) -> p t w", w=128)
                nc.sync.dma_start_transpose(xt0v, x0[:])
                xt1 = xtpool.tile([128, NCHUNK * 128], f16, tag="xt1")
                xt1v = xt1[:].rearrange("p (t w) -> p t w", w=128)
                nc.sync.dma_start_transpose(xt1v, x1[:])

                # U in two half tiles so the DVE transpose of half A can
                # start while exp tiles of half B are still being produced
                u16a = upool.tile([K, N // 2], f16, tag="ua")
                u16b = upool.tile([K, N // 2], f16, tag="ub")

                def u_slice(i):
                    lo = i * TN
                    if lo < N // 2:
                        return u16a[:, lo:lo + TN]
                    return u16b[:, lo - N // 2:lo - N // 2 + TN]

                for g in range(NT // SQG):
                    xsq = sqpool.tile([128, 2 * SQG * TN], f16, tag="xsq")
                    xsq3 = xsq[:].rearrange("p (c m) -> p c m", c=2)
                    sl = bass.ts(g, SQG * TN)
                    nc.vector.tensor_mul(xsq3[:, 0, :], x0[:, sl], x0[:, sl])
                    nc.vector.tensor_mul(xsq3[:, 1, :], x1[:, sl], x1[:, sl])
                    for j in range(SQG):
                        i = g * SQG + j
                        pl = psl.tile([K, TN], f32)
                        xs = bass.ts(i, TN)
                        js = bass.ts(j, TN)
                        nc.tensor.matmul(pl[:], g16[:].rearrange("p (c k) -> p c k", k=K)[:, 0, :],
                                         x0[:, xs], start=True, stop=False, skip_group_check=True)
                        nc.tensor.matmul(pl[:], g16[:].rearrange("p (c k) -> p c k", k=K)[:, 1, :],
                                         x1[:, xs], start=False, stop=False, skip_group_check=True)
                        nc.tensor.matmul(pl[:], s16[:], xsq3[:, 0, js],
                                         start=False, stop=False, skip_group_check=True)
                        nc.tensor.matmul(pl[:], s16[:], xsq3[:, 1, js],
                                         start=False, stop=True, skip_group_check=True)
                        nc.scalar.activation(u_slice(i), pl[:], EXP, bias=bias[:], scale=1.0)

                ut = utpool.tile([128, NCHUNK * K], f16)
                ut3 = ut[:].rearrange("p (t k) -> p t k", k=K)
                # ut3[32j+w, t, k] = U[k, t*128 + 32j + w]:
                # 32x32-block DVE stream-transposes, per j and per U-half
                HT = NCHUNK // 2
                for uh, ut16 in ((0, u16a), (1, u16b)):
                    uv = ut16[:].rearrange("k (t j w) -> k t j w", j=4, w=32)
                    for j in range(4):
                        nc.vector.transpose(
                            ut3[32 * j:32 * (j + 1), uh * HT:(uh + 1) * HT, :],
                            uv[:, :, j, :])

                # per-group A tiles so E-matmuls start after the first
                # normalize group instead of after all 72 chunks
                NG = 4
                pe = pse.tile([K, 257], f32)
                for g in range(NCHUNK // NG):
                    den = npool.tile([128, NG], f32, tag="den")
                    den3 = den[:].rearrange("p (t o) -> p t o", o=1)
                    sl3 = ut3[:, g * NG:(g + 1) * NG, :]
                    nc.vector.reduce_sum(den3, sl3, axis=AX)
                    rec = npool.tile([128, NG], f32, tag="rec")
                    nc.vector.reciprocal(rec[:], den[:])
                    recb = rec[:].rearrange("p (t o) -> p t o", o=1).broadcast_to((128, NG, K))
                    ag = apool.tile([128, NG * K], f16, tag=f"a{g}")
                    ag3 = ag[:].rearrange("p (t k) -> p t k", k=K)
                    nc.vector.tensor_mul(ag3[:], sl3, recb)
                    for tt in range(NG):
                        t = g * NG + tt
                        sp = (t == NCHUNK - 1)
                        # start=True clears has_written for the whole PSUM
                        # bank, so only the very first matmul touching this
                        # bank may set it.
                        nc.tensor.matmul(pe[:, 0:128], ag3[:, tt, :], xt0v[:, t, :],
                                         start=(t == 0), stop=sp, skip_group_check=True)
                        nc.tensor.matmul(pe[:, 128:256], ag3[:, tt, :], xt1v[:, t, :],
                                         start=False, stop=sp, skip_group_check=True)
                        nc.tensor.matmul(pe[:, 256:257], ag3[:, tt, :], ones_col16[:],
                                         start=False, stop=sp, skip_group_check=True)

                cs = opool.tile([K, D], f32, tag=f"cs{b}")
                nc.vector.tensor_scalar_mul(cs[:], cw_sb[:], pe[:, 256:257])
                ef = opool.tile([K, D], f32, tag=f"ef{b}")
                nc.vector.tensor_sub(ef[:], pe[:, 0:256], cs[:])
                efs.append(ef)

            # all output stores after the last transpose: each interleaved
            # DMA around the transpose fence costs a ~3us chain link
            for b, ef in enumerate(efs):
                nc.sync.dma_start(eout[b], ef[:])

    _split_multi_waits(nc)
    return nc


_NC_CACHE = None


def _run(X, codewords, scale, trace=False, tmpdir=None):
    global _NC_CACHE
    if _NC_CACHE is None:
        _NC_CACHE = _build_module()
    nc = _NC_CACHE
    Xr = np.ascontiguousarray(X.reshape(B, D, N), dtype=np.float32)
    cw = np.ascontiguousarray(codewords, dtype=np.float32)
    s = np.asarray(scale, dtype=np.float32).reshape(-1)
    in_maps = []
    for c in range(NCORES):
        in_maps.append({
            "xin": Xr[c * BPC:(c + 1) * BPC],
            "cw": cw,
            "s_col": np.ascontiguousarray(s.reshape(K, 1)),
            "s_row": np.ascontiguousarray(s.reshape(1, K)),
        })
    kr = run_bass_kernel_spmd(nc, in_maps, list(range(NCORES)),
                              trace=trace, tmpdir=tmpdir)
    out = np.concatenate([r["eout"] for r in kr.results], axis=0)
    return out.astype(np.float32), kr


def kernel(X, codewords, scale):
    out, _ = _run(X, codewords, scale)
    return out



# revision 2
# speedup vs baseline: 1.3935x; 1.3935x over previous
"""VQ codebook soft-assignment encoding kernel for 8 trn2 NeuronCores. v2.

Math (per batch b):
  Xf = X[b].reshape(D, N).T                        # [N, D], N = H*W
  logit[n,k] = scale[k] * (||x_n||^2 - 2<x_n,c_k> + ||c_k||^2)
  A = softmax(logit, axis=k)
  E[b,k,:] = sum_n A[n,k] * (x_n - c_k)            # [K, D]

Sharding: data-parallel over B (4 batches per core), codewords/scale replicated.

v2 changes vs baseline: no DMA-xbar transposes at all (they serialized the
entire DMA timeline against the HBM loads). X^T is produced on the PE in
transpose-mode and evacuated PSUM->SBUF on ACT/DVE; logits and E matmuls are
4-way column-tiled (M=K=32 -> 4 concurrent col-groups); exp runs on 128
partitions; the 4 E partial accumulators are folded by one [128,32]^T matmul.
"""
import numpy as np
from contextlib import ExitStack

import concourse.bass as bass
import concourse.mybir as mybir
import concourse.tile as tile
from concourse.tile import ScopedClock
from concourse.bass_utils import run_bass_kernel_spmd

dt = mybir.dt

B, D, K, H, W = 32, 256, 32, 96, 96
N = H * W                 # 9216
NCORES = 8
BPC = B // NCORES         # 4 batches per core
TN = 512                  # n-tile for logits
NT = N // TN              # 18 tiles -> 4 full groups of 4 + 1 half group of 2
NCHUNK = N // 128         # 72 chunks for E-matmul / transposes
XTW = 260                 # per-chunk column stride in XT16 (256 d + ones + pad)
WCH = 8                   # chunks per X-transpose evacuation wave
NWAVE = NCHUNK // WCH     # 9 waves
EVAC_ACT = 9              # leading waves evacuated by ACT (rest DVE), 0..NWAVE
XSQ_DVE = 2048            # x0^2 columns computed on DVE (rest ACT; x1^2 all DVE)


def _patch_tile_drain():
    """This toolchain's walrus allows only one sync-wait per instruction.
    Split the tail drain's waits across chained drains."""
    if getattr(tile.TileContext, "_drain_patched", False):
        return

    def _drain_and_barrier_split(self, tick_clock, wait_clock):
        nc = self.nc
        drain_inst = nc.sync.drain()
        wait_clock.add_sem_waits(
            drain_inst.ins, ScopedClock({None: tick_clock.global_clock})
        )
        si = drain_inst.ins.sync_info
        if si is not None and si.on_wait and len(si.on_wait) > 1:
            extra = list(si.on_wait[1:])
            del si.on_wait[1:]
            for w in extra:
                d = nc.sync.drain()
                dsi = d.ins.sync_info
                if dsi is None:
                    d.ins.sync_info = mybir.SyncInfo(on_wait=[w], on_update=[])
                else:
                    dsi.on_wait.append(w)
        nc.all_engine_barrier()
        assert self.sems is not None
        popped = nc._tile_sem_poison_stack.pop()
        assert popped is self._sem_poison
        nc.clear_and_free_semaphores(list(self.sems.allocated().values()))
        nc.all_engine_barrier()

    tile.TileContext._drain_and_barrier = _drain_and_barrier_split
    tile.TileContext._drain_patched = True


def _split_multi_waits(nc):
    """Hoist extra sem-waits onto standalone event-sem instructions."""
    n_split = 0
    for f in nc.m.functions:
        for bb in f.blocks:
            new_list = []
            for inst in bb.instructions:
                si = inst.sync_info
                if si is not None and si.on_wait is not None and len(si.on_wait) > 1:
                    extra = list(si.on_wait[:-1])
                    keep = [si.on_wait[-1]]
                    for w in extra:
                        ev = mybir.InstEventSemaphore(
                            name=f"{inst.name}-wsplit{n_split}",
                            ins=[], outs=[],
                            sync_info=mybir.SyncInfo(on_wait=[w], on_update=[]),
                        )
                        ev.engine = inst.engine
                        nc.register_instruction(ev)
                        new_list.append(ev)
                        n_split += 1
                    del si.on_wait[:]
                    si.on_wait.extend(keep)
                new_list.append(inst)
            bb.instructions[:] = new_list
    return n_split


def _build_module(bpc=BPC, sim_safe=False):
    _patch_tile_drain()
    nc = bass.Bass()
    xin = nc.declare_dram_parameter("xin", [bpc, D, N], dt.float32, isOutput=False)
    cw = nc.declare_dram_parameter("cw", [K, D], dt.float32, isOutput=False)
    s_col = nc.declare_dram_parameter("s_col", [K, 1], dt.float32, isOutput=False)
    s_row = nc.declare_dram_parameter("s_row", [1, K], dt.float32, isOutput=False)
    id128 = nc.declare_dram_parameter("id128", [128, 128], dt.float16, isOutput=False)
    fold4 = nc.declare_dram_parameter("fold4", [128, K], dt.float16, isOutput=False)
    eout = nc.declare_dram_parameter("eout", [bpc, K, D], dt.float32, isOutput=True)

    f16, f32 = dt.float16, dt.float32
    AX = mybir.AxisListType.X
    EXP = mybir.ActivationFunctionType.Exp
    SQ = mybir.ActivationFunctionType.Square
    CP = mybir.ActivationFunctionType.Copy

    with tile.TileContext(nc) as tc:
        with ExitStack() as ctx:
            singles = ctx.enter_context(tc.tile_pool(name="singles", bufs=1))
            psl = ctx.enter_context(tc.tile_pool(name="psl", bufs=2, space="PSUM"))

            # ---- one-time prep from codewords/scale ----
            cw_sb = singles.tile([K, D], f32)
            nc.sync.dma_start(cw_sb[:], cw[:])
            scol_sb = singles.tile([K, 1], f32)
            nc.sync.dma_start(scol_sb[:], s_col[:])
            srow_sb = singles.tile([1, K], f32)
            nc.sync.dma_start(srow_sb[:], s_row[:])
            id16 = singles.tile([128, 128], f16)
            nc.sync.dma_start(id16[:], id128[:])
            fold16 = singles.tile([128, K], f16)
            nc.sync.dma_start(fold16[:], fold4[:])

            # G16 [128, 2, K]: G[p, c, k] = -2 s_k c[k, c*128+p]
            w1 = singles.tile([K, D], f32)
            nc.vector.tensor_scalar_mul(w1[:], cw_sb[:], scol_sb[:])
            w2 = singles.tile([K, D], f32)
            nc.vector.tensor_scalar_mul(w2[:], w1[:], -2.0)
            w16 = singles.tile([K, D], f16)
            nc.vector.tensor_copy(w16[:], w2[:])
            g16 = singles.tile([128, 2 * K], f16)
            g3t = g16[:].rearrange("p (c k) -> p c k", k=K)
            wv = w16[:].rearrange("k (c j w) -> k c j w", j=4, w=32)
            for j in range(4):
                if sim_safe:
                    for c in range(2):
                        nc.vector.transpose(g3t[32 * j:32 * (j + 1), c, :],
                                            wv[:, c, j, :])
                else:
                    nc.vector.transpose(g3t[32 * j:32 * (j + 1), :, :],
                                        wv[:, :, j, :])

            # S16 [128, K]: every row = s_k (fp16)
            ones_row16 = singles.tile([1, 128], f16)
            nc.vector.memset(ones_row16[:], 1.0)
            ones_col16 = singles.tile([128, 1], f16)
            nc.vector.memset(ones_col16[:], 1.0)
            srow16 = singles.tile([1, K], f16)
            nc.vector.tensor_copy(srow16[:], srow_sb[:])
            ps_s = psl.tile([128, TN], f32, tag="psl")
            nc.tensor.matmul(ps_s[:, 0:K], ones_row16[:], srow16[:], start=True,
                             stop=True, skip_group_check=True)
            s16 = singles.tile([128, K], f16)
            nc.vector.tensor_copy(s16[:], ps_s[:, 0:K])

            # bias128 [128, 1]: bias[32j + k] = s_k * ||c_k||^2 (4 stripes)
            csq = singles.tile([K, D], f32)
            nc.vector.tensor_mul(csq[:], cw_sb[:], cw_sb[:])
            sqc = singles.tile([K, 1], f32)
            nc.vector.reduce_sum(
                sqc[:].rearrange("k (o p) -> k o p", o=1),
                csq[:].rearrange("k (o d) -> k o d", o=1), axis=AX)
            bias = singles.tile([K, 1], f32)
            nc.vector.tensor_mul(bias[:], sqc[:], scol_sb[:])
            bias128 = singles.tile([128, 1], f32)
            for j in range(4):
                nc.vector.tensor_copy(bias128[32 * j:32 * (j + 1), :], bias[:])

            # ---- per-batch pools ----
            xpool = ctx.enter_context(tc.tile_pool(name="x16", bufs=2))
            sqpool = ctx.enter_context(tc.tile_pool(name="xsq", bufs=1))
            upool = ctx.enter_context(tc.tile_pool(name="u16", bufs=2))
            utpool = ctx.enter_context(tc.tile_pool(name="ut16", bufs=2))
            npool = ctx.enter_context(tc.tile_pool(name="nrm", bufs=4))
            atpool = ctx.enter_context(tc.tile_pool(name="at16", bufs=2))
            xtpool = ctx.enter_context(tc.tile_pool(name="xt16", bufs=1))
            fpool = ctx.enter_context(tc.tile_pool(name="f16", bufs=2))
            opool = ctx.enter_context(tc.tile_pool(name="out", bufs=2))
            psxt = ctx.enter_context(tc.tile_pool(name="psxt", bufs=2, space="PSUM"))
            pse = ctx.enter_context(tc.tile_pool(name="pse", bufs=2, space="PSUM"))

            g16v = g16[:].rearrange("p (c k) -> p c k", k=K)

            def batch_tail(pe, b):
                # fold 4 partials + E = psF[:, :256] - S_k c; emitted at the
                # TOP of the next slot so no engine queue blocks on E(b)
                ef16 = fpool.tile([128, 257], f16, tag="f")
                nc.scalar.activation(ef16[:], pe[:, 0:257], CP)
                psf = psl.tile([128, TN], f32, tag="psl")
                nc.tensor.matmul(psf[0:K, 0:257], fold16[:], ef16[:],
                                 start=True, stop=True, skip_group_check=True)
                cs = opool.tile([K, D], f32, tag="cs")
                nc.vector.tensor_scalar_mul(cs[:], cw_sb[:], psf[0:K, 256:257])
                ef = opool.tile([K, D], f32, tag="ef")
                nc.vector.tensor_sub(ef[:], psf[0:K, 0:256], cs[:])
                nc.sync.dma_start(eout[b], ef[:])

            prev = None
            for b in range(bpc):
                x0 = xpool.tile([128, N], f16, tag="x0")
                nc.gpsimd.dma_start(x0[:], xin[b, 0:128, :])
                x1 = xpool.tile([128, N], f16, tag="x1")
                nc.gpsimd.dma_start(x1[:], xin[b, 128:256, :])
                if prev is not None:
                    batch_tail(*prev)

                # U16 [128, 5*512]: group g cols [512g, 512g+512);
                # partition 32j+k holds n-tile t=4g+j (g=4: j in {0,1}).
                u16 = upool.tile([128, 5 * TN], f16, tag="u")

                # x^2 up front in big calls: xsq[:, 0:N] = x0^2 (DVE head,
                # ACT tail), xsq[:, N:2N] = x1^2 (DVE)
                xsq = sqpool.tile([128, 2 * N], f16, tag="xsq")
                nc.vector.tensor_mul(xsq[:, 0:XSQ_DVE], x0[:, 0:XSQ_DVE],
                                     x0[:, 0:XSQ_DVE])
                nc.scalar.activation(xsq[:, XSQ_DVE:N], x0[:, XSQ_DVE:N], SQ)
                nc.vector.tensor_mul(xsq[:, N:2 * N], x1[:], x1[:])

                # ---- logits (col-tiled) + exp, per group of 4 n-tiles ----
                for g in range(5):
                    nj = 4 if g < 4 else 2
                    pl = psl.tile([128, TN], f32, tag="psl")
                    for j in range(nj):
                        t = 4 * g + j
                        ts = bass.ts(t, TN)
                        ps_j = pl[32 * j:32 * (j + 1), :]
                        tp = (0, 32 * j)
                        nc.tensor.matmul(ps_j, g16v[:, 0, :], x0[:, ts],
                                         start=True, stop=False,
                                         skip_group_check=True, tile_position=tp)
                        nc.tensor.matmul(ps_j, g16v[:, 1, :], x1[:, ts],
                                         start=False, stop=False,
                                         skip_group_check=True, tile_position=tp)
                        nc.tensor.matmul(ps_j, s16[:], xsq[:, ts],
                                         start=False, stop=False,
                                         skip_group_check=True, tile_position=tp)
                        nc.tensor.matmul(ps_j, s16[:], xsq[:, N + 512 * t:
                                                           N + 512 * (t + 1)],
                                         start=False, stop=True,
                                         skip_group_check=True, tile_position=tp)
                    npart = 32 * nj
                    nc.scalar.activation(u16[0:npart, bass.ts(g, TN)],
                                         pl[0:npart, :], EXP,
                                         bias=bias128[0:npart, :], scale=1.0)

                # ---- U^T on DVE: UT16 [128, 72*32], col = 32*tt + k ----
                # tt = 16g + 4j + q (g<4), tt = 64 + 4j + q (g=4, j<2)
                ut16 = utpool.tile([128, NCHUNK * K], f16, tag="ut")
                utf = ut16[:, 0:2048].rearrange("p (g j q k) -> p g j q k",
                                                g=4, j=4, q=4)
                uf = u16[:, 0:2048].rearrange("p (g q a w) -> p g q a w",
                                              g=4, q=4, a=4)
                for j in range(4):
                    for a in range(4):
                        if sim_safe:
                            for g in range(4):
                                for q in range(4):
                                    nc.vector.transpose(
                                        utf[32 * a:32 * (a + 1), g, j, q, :],
                                        uf[32 * j:32 * (j + 1), g, q, a, :])
                        else:
                            nc.vector.transpose(
                                utf[32 * a:32 * (a + 1), :, j, :, :],
                                uf[32 * j:32 * (j + 1), :, :, a, :])
                uth = ut16[:, 2048:2304].rearrange("p (j q k) -> p j q k", j=2, q=4)
                uh = u16[:, 2048:2560].rearrange("p (q a w) -> p q a w", q=4, a=4)
                for j in range(2):
                    for a in range(4):
                        if sim_safe:
                            for q in range(4):
                                nc.vector.transpose(
                                    uth[32 * a:32 * (a + 1), j, q, :],
                                    uh[32 * j:32 * (j + 1), q, a, :])
                        else:
                            nc.vector.transpose(
                                uth[32 * a:32 * (a + 1), j, :, :],
                                uh[32 * j:32 * (j + 1), :, a, :])

                # den, recip, A^T = U^T * (1/den): emitted in halves,
                # interleaved with the E-matmul halves further below
                at16 = atpool.tile([128, NCHUNK * K], f16, tag="at")
                den = npool.tile([128, NCHUNK], f32, tag="den")
                rec = npool.tile([128, NCHUNK], f32, tag="rec")
                rec16 = npool.tile([128, NCHUNK], f16, tag="rec16")
                HC = NCHUNK // 2

                def norm_half(h):
                    hs = slice(h * HC, (h + 1) * HC)
                    hks = slice(h * HC * K, (h + 1) * HC * K)
                    nc.vector.reduce_sum(
                        den[:, hs].rearrange("p (t o) -> p t o", o=1),
                        ut16[:, hks].rearrange("p (t k) -> p t k", k=K), axis=AX)
                    nc.vector.reciprocal(rec[:, hs], den[:, hs])
                    nc.vector.tensor_copy(rec16[:, hs], rec[:, hs])
                    recb = rec16[:, hs].rearrange("p (t o) -> p t o", o=1)
                    recb = recb.broadcast_to((128, HC, K))
                    nc.vector.tensor_mul(
                        at16[:, hks].rearrange("p (t k) -> p t k", k=K),
                        ut16[:, hks].rearrange("p (t k) -> p t k", k=K), recb)

                # ---- X^T on PE (transpose mode), evac ACT/DVE ----
                # XT16 [128, 72*260]: chunk t cols [260t, 260t+256) = x[n,:],
                # col 260t+256 = 1.0
                xt16 = xtpool.tile([128, NCHUNK * XTW], f16, tag="xt")
                xt3 = xt16[:].rearrange("p (t c) -> p t c", c=XTW)
                nc.vector.memset(xt3[:, :, 256:257], 1.0)
                for w in range(NWAVE):
                    pw = psxt.tile([128, WCH * 256], f16, tag="psxt")
                    for c in range(WCH):
                        t = WCH * w + c
                        ts = bass.ts(t, 128)
                        nc.tensor.transpose(pw[:, 256 * c:256 * c + 128],
                                            x0[:, ts], id16[:])
                        nc.tensor.transpose(pw[:, 256 * c + 128:256 * (c + 1)],
                                            x1[:, ts], id16[:])
                    dst = xt3[:, WCH * w:WCH * (w + 1), 0:256]
                    src = pw[:].rearrange("p (t c) -> p t c", c=256)
                    if w < EVAC_ACT:
                        nc.scalar.activation(dst, src, CP)
                    else:
                        nc.vector.tensor_copy(dst, src)

                # ---- E-matmul: 4 col-tiled partial accumulators, emitted in
                # halves so E chunks 0..35 overlap the second normalize half
                pe = pse.tile([128, TN], f32, tag="pse")
                for h in range(2):
                    norm_half(h)
                    for t in range(h * HC, (h + 1) * HC):
                        j = t % 4
                        nc.tensor.matmul(pe[32 * j:32 * (j + 1), 0:257],
                                         at16[:, bass.ts(t, K)],
                                         xt16[:, XTW * t:XTW * t + 257],
                                         start=(t < 4), stop=(t >= NCHUNK - 4),
                                         skip_group_check=True,
                                         tile_position=(0, 32 * j))

                prev = (pe, b)
            batch_tail(*prev)

    _split_multi_waits(nc)
    return nc


_NC_CACHE = None


def _run(X, codewords, scale, trace=False, tmpdir=None):
    global _NC_CACHE
    if _NC_CACHE is None:
        _NC_CACHE = _build_module()
    nc = _NC_CACHE
    Xr = np.ascontiguousarray(X.reshape(B, D, N), dtype=np.float32)
    cw = np.ascontiguousarray(codewords, dtype=np.float32)
    s = np.asarray(scale, dtype=np.float32).reshape(-1)
    id128 = np.eye(128, dtype=np.float16)
    fold4 = np.zeros((128, K), dtype=np.float16)
    for p in range(128):
        fold4[p, p % K] = 1.0
    in_maps = []
    for c in range(NCORES):
        in_maps.append({
            "xin": Xr[c * BPC:(c + 1) * BPC],
            "cw": cw,
            "s_col": np.ascontiguousarray(s.reshape(K, 1)),
            "s_row": np.ascontiguousarray(s.reshape(1, K)),
            "id128": id128,
            "fold4": fold4,
        })
    kr = run_bass_kernel_spmd(nc, in_maps, list(range(NCORES)),
                              trace=trace, tmpdir=tmpdir)
    out = np.concatenate([r["eout"] for r in kr.results], axis=0)
    return out.astype(np.float32), kr


def kernel(X, codewords, scale):
    out, _ = _run(X, codewords, scale)
    return out
